# revision 32
# baseline (speedup 1.0000x reference)
"""4D circular cross-correlation (qcd_ml C_Convolution, k=3, nd=4) on 8 TRN2 cores.

Math: out[o, x,y,z,t, s,c] = b[o] + sum_{i, ax,ay,az,at} W[i,o,ax,ay,az,at]
                                   * U[i, x+ax-1, y+ay-1, z+az-1, t+at-1, s,c]
(all site indices circular). U complex64 (4,16,16,16,32,4,3), W complex64
(4,4,3,3,3,3), b complex64 (4,).

Device mapping (per core, T sharded 8-way with +-1 halos prepared on host):
  - contraction (matmul partition) dim = (reim_in 2, C_in 4, X 16) = 128
  - output (PSUM partition) dim       = (reim_out 2, C_out 4, X0 16) = 128
  - X offsets (ax) live inside the stationary 128x128 matrices, circularly
    banded in (x, x0); complex arithmetic is the 2x2 [[Wr, Wi], [-Wi, Wr]]
    block over the reim axes.
  - The T offsets (at) are removed by a host-side Winograd F(4,3) transform
    along t (6 phases per 4 local t outputs).

Schedule (phase-major, stationary-reuse):
  - Phases processed sequentially over the whole local lattice in order
    [1,2,3,4,5,0]; per phase, the 9 (ay,az) stationaries are each loaded
    once and applied to all 8 y-pairs back-to-back (9 LDWEIGHTS per phase,
    72 matmuls), into 8 PSUM banks (one per y-pair).
  - U~ is laid out phase-major in DRAM so the first phase's slab (plus its
    9 stationaries) lands after ~0.5 MB of DMA -> PE starts within ~2 us.
  - The inverse Winograd transform A^T is accumulated incrementally: each
    phase's PSUM tile is drained once by the Activation engine into an SBUF
    stage, then 4 scalar_tensor_tensor accumulations (split DVE/Pool) add
    stage * AT[r,p] into the 4 per-pair output slots. Phases 5 and 0 touch
    a single slot and are accumulated straight from PSUM.
  - Output is slot-major [128, TLOC, Y, Z, SC] so each (pair, t-slot)
    finishes early and is stored as its own contiguous 196 KB DMA; only the
    final pair's slot-0 store is exposed at the tail.
  - bf16 for U~ and stationaries (PSUM accumulates fp32); fp32 elsewhere.
"""

import os
import sys
import itertools
import numpy as np

for _p in ("/opt/trn_rl_repo",):
    if _p not in sys.path and os.path.isdir(_p):
        sys.path.insert(0, _p)

C_IN, C_OUT = 4, 4
X = Y = Z = 16
T = 32
SC = 12  # spin*color
NCORES = 8
TLOC = T // NCORES          # 4 = one F(4,3) output tile
NPH = 6                     # Winograd F(4,3) phases
YPAD, ZPAD = Y + 2, Z + 2   # 18
OFF9 = list(itertools.product(range(3), repeat=2))  # (ay, az)
NPAIR = Y // 2              # 8 y-pairs
FREE = 2 * Z * SC           # 384 free elems per matmul

USE_BF16 = os.environ.get("CONV_BF16", "1") == "1"

# Winograd F(4,3), points [0,1,-1,2,-2,inf] (correlation form:
# out[r] = sum_k g[k] d[r+k], r=0..3, d = U[t0-1 .. t0+4]).
BT = np.array([
    [4, 0, -5, 0, 1, 0],
    [0, -4, -4, 1, 1, 0],
    [0, 4, -4, -1, 1, 0],
    [0, -2, -1, 2, 1, 0],
    [0, 2, -1, -2, 1, 0],
    [0, 4, 0, -5, 0, 1]], np.float64)
G = np.array([
    [1 / 4, 0, 0],
    [-1 / 6, -1 / 6, -1 / 6],
    [-1 / 6, 1 / 6, -1 / 6],
    [1 / 24, 1 / 12, 1 / 6],
    [1 / 24, -1 / 12, 1 / 6],
    [0, 0, 1]], np.float64)
# A^T[r, p]; columns indexed by phase p.
AT = np.array([
    [1, 1, 1, 1, 1, 0],
    [0, 1, -1, 2, -2, 0],
    [0, 1, 1, 4, 4, 0],
    [0, 1, -1, 8, -8, 1]], np.float64)
# Processing order: the two single-slot phases (5 -> slot 3, 0 -> slot 0)
# go last so each phase's PSUM drain is one op and the tail is minimal.
PH_ORDER = [1, 2, 3, 4, 5, 0]


def _prep_u_shards(U):
    """U complex (4,16,16,16,32,4,3) -> per-core float32 arrays
    [128, NPH, YPAD, ZPAD, SC] of the t-Winograd-transformed field
    (phase-major for consumption-ordered streaming)."""
    Ur = np.stack([U.real, U.imag], axis=0).astype(np.float32)  # (2,4,X,Y,Z,T,4,3)
    Ur = Ur.reshape(2, C_IN, X, Y, Z, T, SC)
    Up = np.pad(Ur, ((0, 0), (0, 0), (0, 0), (1, 1), (1, 1), (0, 0), (0, 0)),
                mode="wrap")  # (2,4,16,18,18,32,12)
    shards = []
    for k in range(NCORES):
        t0 = k * TLOC
        tidx = np.arange(t0 - 1, t0 + 5) % T        # 6-point window
        d = np.take(Up, tidx, axis=5)               # (2,4,16,18,18,6,12)
        m = np.einsum("pk,rixyzks->rixyzps", BT,
                      d.astype(np.float64)).astype(np.float32)
        # (2,4,16,18,18,6,12) -> (2,4,16, 6, 18, 18, 12)
        m = m.transpose(0, 1, 2, 5, 3, 4, 6)
        m = m.reshape(128, NPH, YPAD, ZPAD, SC)
        shards.append(np.ascontiguousarray(m))
    return shards


def _prep_wstat(W):
    """W complex (4,4,3,3,3,3) -> [128, NPH, 9, 128] float32 stationary stack.

    For phase p and (ay,az): Wg[p][i,o,ax,ay,az] = sum_at G[p,at] W[..,at];
    band in (x,x0): ax = (x - x0 + 1) mod 16 in {0,1,2};
    ri block M = [[Wr, Wi], [-Wi, Wr]] (columns riO: out_r, out_i).
    """
    Wc = np.ascontiguousarray(W).astype(np.complex128)
    Wg = np.einsum("pk,ioxyzk->pioxyz", G.astype(np.complex128), Wc)
    Wg = Wg.astype(np.complex64)                    # (6,4,4,3,3,3)
    stat = np.zeros((2, C_IN, X, NPH, 9, 2, C_OUT, X), np.float32)
    for ph in range(NPH):
        for aidx, (ay, az) in enumerate(OFF9):
            for ax in range(3):
                wr = Wg[ph, :, :, ax, ay, az].real
                wi = Wg[ph, :, :, ax, ay, az].imag
                for x0 in range(X):
                    x = (x0 + ax - 1) % X
                    stat[0, :, x, ph, aidx, 0, :, x0] = wr
                    stat[1, :, x, ph, aidx, 0, :, x0] = -wi
                    stat[0, :, x, ph, aidx, 1, :, x0] = wi
                    stat[1, :, x, ph, aidx, 1, :, x0] = wr
    return np.ascontiguousarray(stat.reshape(128, NPH, 9, 128))


def _assemble(results, b):
    """results[k]["out"]: [128, TLOC, Y, Z, SC] f32 -> complex (4,16,16,16,32,4,3)."""
    out = np.empty((C_OUT, X, Y, Z, T, SC), np.complex64)
    for k in range(NCORES):
        r = np.asarray(results[k]["out"], np.float32).reshape(
            2, C_OUT, X, TLOC, Y, Z, SC)
        # (o, x, tr, y, z, s) -> (o, x, y, z, tr, s)
        rr = r[0] + 1j * r[1]
        out[:, :, :, :, k * TLOC:(k + 1) * TLOC, :] = rr.transpose(0, 1, 3, 4, 2, 5)
    out += np.asarray(b, np.complex64).reshape(C_OUT, 1, 1, 1, 1, 1)
    return np.ascontiguousarray(out.reshape(C_OUT, X, Y, Z, T, 4, 3))


def _build_nc():
    import concourse.mybir as mybir
    from concourse import bacc, tile
    from contextlib import ExitStack

    f32 = mybir.dt.float32
    in_dt = mybir.dt.bfloat16 if USE_BF16 else mybir.dt.float32r
    out_dt = mybir.dt.bfloat16 if USE_BF16 else f32
    AluOp = mybir.AluOpType

    nc = bacc.Bacc()
    w_dram = nc.declare_dram_parameter("wstat", [128, NPH, 9, 128], in_dt,
                                       isOutput=False)
    u_dram = nc.declare_dram_parameter("u", [128, NPH, YPAD, ZPAD, SC], in_dt,
                                       isOutput=False)
    o_dram = nc.declare_dram_parameter("out", [128, TLOC, NPAIR, 2, Z, SC],
                                       out_dt, isOutput=True)

    with tile.TileContext(nc) as tc, ExitStack() as ctx:
        ipool = ctx.enter_context(tc.tile_pool(name="inp", bufs=1))
        spool = ctx.enter_context(tc.tile_pool(name="stg", bufs=16))
        opool = ctx.enter_context(tc.tile_pool(name="ot", bufs=1))
        ppool = ctx.enter_context(tc.tile_pool(name="psum", bufs=1, space="PSUM"))

        wt = ipool.tile([128, NPH, 9, 128], in_dt, tag="w")
        uf = ipool.tile([128, NPH, YPAD, ZPAD, SC], in_dt, tag="u")
        # PE warmup fodder: zeroed stationary + rhs.
        wz = ipool.tile([128, 128], in_dt, tag="wz")
        rz = ipool.tile([128, FREE], in_dt, tag="rz")
        nc.vector.memset(wz[:], 0)
        nc.vector.memset(rz[:], 0)

        # Consumption-ordered streaming (the HWDGE ring is FIFO per issuing
        # engine, and each dma_start trigger costs ~0.6 us on Sync -- keep
        # the trigger count low and the first phase's data up front).
        p0 = PH_ORDER[0]
        nc.sync.dma_start(wt[:, p0:p0 + 1], w_dram[:, p0:p0 + 1])
        for r0, r1 in ((0, 4), (4, 8), (8, 12), (12, 16), (16, YPAD)):
            nc.sync.dma_start(uf[:, p0, r0:r1], u_dram[:, p0, r0:r1])
        for p in PH_ORDER[1:]:
            nc.sync.dma_start(wt[:, p:p + 1], w_dram[:, p:p + 1])
            nc.sync.dma_start(uf[:, p, 0:9], u_dram[:, p, 0:9])
            nc.sync.dma_start(uf[:, p, 9:YPAD], u_dram[:, p, 9:YPAD])

        # Merged output accumulator [128, TLOC, NPAIR, 2, Z, SC] (same
        # linear layout as [128, TLOC, Y, Z, SC]); one-shot slot writes.
        ot = opool.tile([128, TLOC, NPAIR, 2, Z, SC], out_dt, tag="ot")

        # Merged whole-lattice staging tiles [128, NPAIR, 2, Z, SC]: the
        # butterfly runs as ONE DVE op per step (3072 free elems) instead
        # of 8 per-pair ops -- per-op overhead (~0.5 us) dominated the
        # per-pair version. Activation does all per-pair PSUM drains.
        def stg(nm):
            return spool.tile([128, NPAIR, 2, Z, SC], out_dt, tag=nm,
                              name=nm, bufs=1)

        c1, c2, c3, c4 = stg("c1"), stg("c2"), stg("c3"), stg("c4")
        c0 = stg("c0")
        bb, aa = stg("b"), stg("a")
        uu, ss = stg("u"), stg("s")
        vv, ww = stg("v"), stg("w")

        def stt(out_ap, in0, scalar, in1, sub=False):
            nc.vector.scalar_tensor_tensor(
                out_ap, in0=in0, scalar=scalar, in1=in1,
                op0=AluOp.mult, op1=AluOp.subtract if sub else AluOp.add)

        # Warm the PE (pstate ramp + HAM) during the framework preamble /
        # initial DMA: one accumulation group of back-to-back matmuls into
        # pair 0's bank, retired before the first real group starts.
        warm = ppool.tile([128, 2, Z, SC], f32, tag="ps0", name="ps0")
        for i in range(12):
            nc.tensor.matmul(warm[:], wz[:], rz[:], start=(i == 0),
                             stop=(i == 11))

        # Butterfly A^T combine, spread across phases:
        #   b=m1+m2, a=m1-m2, u=m3+m4, s=m3-m4, v=b+u, w=a+8s
        #   t0=m0+v, t1=a+2s, t2=b+4u, t3=w+m5
        # Act frees each PSUM bank within ~0.6 us of its group finishing;
        # chain-end slots (t3, t0) drain PSUM directly on DVE.
        for k, p in enumerate(PH_ORDER):
            pts = [ppool.tile([128, 2, Z, SC], f32, tag=f"ps{j}",
                              name=f"ps{j}")
                   for j in range(NPAIR)]
            # Stationary-reuse: one LDWEIGHTS per (p, aidx), 8 matmuls.
            # Order is strictly uniform across windows (aidx-major): any
            # mixed ordering measurably de-optimizes the whole program's
            # sync; the later stream start (full first slab needed) is
            # cheaper than pair-major's exposed weight reloads.
            order = [(j, aidx) for aidx in range(len(OFF9))
                     for j in range(NPAIR)]
            for j, aidx in order:
                ay, az = OFF9[aidx]
                y = 2 * j
                rhs = uf[:, p, y + ay: y + ay + 2, az: az + Z, :]
                nc.tensor.matmul(pts[j][:], wt[:, p, aidx, :], rhs,
                                 start=(aidx == 0), stop=(aidx == 8))
            H = NPAIR // 2  # half-lattice op granularity: latency/overlap balance
            if p == 1:
                for j in range(NPAIR):
                    nc.scalar.copy(c1[:, j], pts[j][:])
            elif p == 2:
                for j in range(NPAIR):
                    nc.scalar.copy(c2[:, j], pts[j][:])
                for h in (0, 1):
                    s_ = slice(h * H, h * H + H)
                    nc.vector.tensor_add(bb[:, s_], c1[:, s_], c2[:, s_])
                for h in (0, 1):
                    s_ = slice(h * H, h * H + H)
                    stt(aa[:, s_], c1[:, s_], 2.0, bb[:, s_], sub=True)
            elif p == 3:
                for j in range(NPAIR):
                    nc.scalar.copy(c3[:, j], pts[j][:])
            elif p == 4:
                for j in range(NPAIR):
                    nc.scalar.copy(c4[:, j], pts[j][:])
                # urgent chain only (t3 needs w): u -> s -> w, in halves
                for h in (0, 1):
                    s_ = slice(h * H, h * H + H)
                    nc.vector.tensor_add(uu[:, s_], c3[:, s_], c4[:, s_])
                for h in (0, 1):
                    s_ = slice(h * H, h * H + H)
                    stt(ss[:, s_], c3[:, s_], 2.0, uu[:, s_], sub=True)
                for h in (0, 1):
                    s_ = slice(h * H, h * H + H)
                    stt(ww[:, s_], ss[:, s_], 8.0, aa[:, s_])
            elif p == 5:
                # PSUM-freeing first: t3 per pair as each m5 group lands.
                for j in range(NPAIR):
                    nc.vector.tensor_add(ot[:, 3, j], ww[:, j], pts[j][:])
                    nc.sync.dma_start(o_dram[:, 3, j], ot[:, 3, j])
                # deferred non-urgent tail (runs on DVE behind the t3s)
                for h in (0, 1):
                    s_ = slice(h * H, h * H + H)
                    nc.vector.tensor_add(vv[:, s_], bb[:, s_], uu[:, s_])
                for h in (0, 1):
                    s_ = slice(h * H, h * H + H)
                    stt(ot[:, 1, s_], ss[:, s_], 2.0, aa[:, s_])
                nc.sync.dma_start(o_dram[:, 1], ot[:, 1])
                for h in (0, 1):
                    s_ = slice(h * H, h * H + H)
                    stt(ot[:, 2, s_], uu[:, s_], 4.0, bb[:, s_])
                nc.sync.dma_start(o_dram[:, 2], ot[:, 2])
            else:  # p == 0, last: Act frees each bank fast; t0 merged after
                for j in range(NPAIR):
                    nc.scalar.copy(c0[:, j], pts[j][:])
                for h in (0, 1):
                    s_ = slice(h * H, h * H + H)
                    nc.vector.tensor_add(ot[:, 0, s_], vv[:, s_], c0[:, s_])
                    nc.sync.dma_start(o_dram[:, 0, s_], ot[:, 0, s_])

    nc.finalize()
    return nc


_NC_CACHE = None
LAST_RUN = None  # BassKernelResults of the most recent device run (for test.py)


def kernel(U, W, b):
    global _NC_CACHE, LAST_RUN
    shards = _prep_u_shards(np.asarray(U))
    wstat = _prep_wstat(np.asarray(W))

    if os.environ.get("CONV_EMULATE", "0") == "1":
        results = _emulate(shards, wstat)
    else:
        from concourse.bass_utils import run_bass_kernel_spmd
        import ml_dtypes

        if _NC_CACHE is None:
            _NC_CACHE = _build_nc()
        np_dt = ml_dtypes.bfloat16 if USE_BF16 else np.float32
        wr = np.ascontiguousarray(wstat.astype(np_dt))
        in_maps = [{"wstat": wr, "u": np.ascontiguousarray(u.astype(np_dt))}
                   for u in shards]
        trace = os.environ.get("CONV_TRACE", "0") == "1"
        LAST_RUN = run_bass_kernel_spmd(
            _NC_CACHE, in_maps, core_ids=list(range(NCORES)), trace=trace)
        results = LAST_RUN.results
    return _assemble(results, np.asarray(b))


def _emulate(shards, wstat):
    """Host-side emulation of the device program (with optional bf16
    quantization of the matmul inputs to model device precision)."""
    if USE_BF16:
        import ml_dtypes
        q = lambda a: a.astype(ml_dtypes.bfloat16).astype(np.float64)
    else:
        q = lambda a: a.astype(np.float64)
    results = []
    for u in shards:
        uq = q(u)
        wq = q(wstat)
        out = np.zeros((128, TLOC, Y, Z, SC), np.float64)
        for j in range(NPAIR):
            y = 2 * j
            for p in range(NPH):
                acc = np.zeros((128, FREE), np.float64)
                for aidx, (ay, az) in enumerate(OFF9):
                    slab = uq[:, p, y + ay: y + ay + 2, az:az + Z, :].reshape(128, -1)
                    acc += wq[:, p, aidx, :].T @ slab
                m = acc.astype(np.float32).reshape(128, 2, Z, SC)
                for r in range(TLOC):
                    out[:, r, y:y + 2] += AT[r, p] * m
        results.append({"out": out.reshape(128, TLOC, Y, Z, SC)})
    return results


if __name__ == "__main__":
    pass
